# revision 1
# baseline (speedup 1.0000x reference)
"""Trainium2 Bass kernel for PointTactileTokenizer (retrieval_knn).

Contract: kernel(**inputs) takes the FULL unsharded inputs (numpy arrays, keys
as in setup_inputs) and returns the FULL output [B, 1+N+M, D] float32.

Strategy: data-parallel over batch B=8 across the 8 NeuronCores; one batch
element per core.  Per core:
  - point/tactile token MLPs in bf16 on the TensorEngine (feature-major)
  - kNN via a rank-score matmul  s[n,m] = [p,1]·[t,-|t|^2/2]  (f32r) so that
    the 8 largest scores == the 8 smallest distances; DVE Max/MaxIndex
    instructions produce exact per-row top-8 values+indices
  - token gather via GPSIMD dma_gather from an on-device row-major bf16 table
  - softmax(-d/T) weights; weighted sum via identity-lhsT matmuls into PSUM
Host does: positional-encoding concat/transposes, weight/bias prep (ctx_emb is
folded into biases / the gather table), final [D,T] -> [T,D] transposes.
"""

import numpy as np
import ml_dtypes

B, N, M, D = 8, 8192, 2048, 256
POINT_FEAT, TAC_FEAT = 6, 16
PE_BANDS, PE_MAX_FREQ = 6, 10.0
K_TACTILE, TAC_TEMP = 8, 0.05
IN_POINT = POINT_FEAT + 3 * 2 * PE_BANDS + 3 * 32   # 138
IN_TAC = TAC_FEAT + 3 * 2 * PE_BANDS                # 52
NT = N // 128                                        # 64 point tiles
BF16 = ml_dtypes.bfloat16

_NC_CACHE = {}


def _pe3_np(xyz):
    freqs = np.linspace(1.0, PE_MAX_FREQ, PE_BANDS, dtype=np.float32)
    x = xyz[..., None] * freqs * np.float32(np.pi)
    pe = np.concatenate([np.sin(x), np.cos(x)], axis=-1)
    return pe.reshape(xyz.shape[0], -1).astype(np.float32)


def _build_nc(repeat=None):
    import os
    if repeat is None:
        repeat = int(os.environ.get("KERNEL_REPEAT", "1"))
    import concourse.bass as bass
    import concourse.tile as tile
    from concourse import library_config
    from concourse import mybir
    from contextlib import ExitStack

    dt = mybir.dt
    AF = mybir.ActivationFunctionType
    ALU = mybir.AluOpType
    AX = mybir.AxisListType

    f32, bf, f32r, i16, u16 = dt.float32, dt.bfloat16, dt.float32r, dt.int16, dt.uint16

    nc = bass.Bass(num_swdge_queues=4)

    # ---- external inputs (per-core shard) ----
    pinA = nc.declare_dram_parameter("pinA", [128, N], bf, isOutput=False)
    pinB = nc.declare_dram_parameter("pinB", [IN_POINT - 128, N], bf, isOutput=False)
    tin = nc.declare_dram_parameter("tin", [IN_TAC, M], bf, isOutput=False)
    pt4 = nc.declare_dram_parameter("pt4", [4, N], f32r, isOutput=False)
    tt4 = nc.declare_dram_parameter("tt4", [4, M], f32r, isOutput=False)
    pnw = nc.declare_dram_parameter("pnw", [128, NT], f32, isOutput=False)
    Wp1a = nc.declare_dram_parameter("Wp1a", [128, D], bf, isOutput=False)
    Wp1b = nc.declare_dram_parameter("Wp1b", [IN_POINT - 128, D], bf, isOutput=False)
    Wp2 = nc.declare_dram_parameter("Wp2", [D, D], bf, isOutput=False)
    Wp3 = nc.declare_dram_parameter("Wp3", [D, D], bf, isOutput=False)
    Wt1 = nc.declare_dram_parameter("Wt1", [IN_TAC, D], bf, isOutput=False)
    Wt2 = nc.declare_dram_parameter("Wt2", [D, D], bf, isOutput=False)
    Wt3 = nc.declare_dram_parameter("Wt3", [D, D], bf, isOutput=False)
    # biases wrapped [128, 2]: chunk c of 128 at column c
    bp1w = nc.declare_dram_parameter("bp1w", [128, 2], f32, isOutput=False)
    bp2w = nc.declare_dram_parameter("bp2w", [128, 2], f32, isOutput=False)
    bt1w = nc.declare_dram_parameter("bt1w", [128, 2], f32, isOutput=False)
    bt2w = nc.declare_dram_parameter("bt2w", [128, 2], f32, isOutput=False)
    btow = nc.declare_dram_parameter("btow", [128, 2], f32, isOutput=False)   # bt3+ctx
    btabw = nc.declare_dram_parameter("btabw", [128, 2], f32, isOutput=False)  # bt3+ctx+bp3
    ident = nc.declare_dram_parameter("ident", [128, 128], bf, isOutput=False)

    out = nc.declare_dram_parameter("out", [D, N + M], f32, isOutput=True)

    # ---- internal DRAM ----
    ttok_rm = nc.dram_tensor("ttok_rm", [M, D], bf)        # gather table (row major)
    idxd = nc.dram_tensor("idxd", [NT, 128, 8], i16)       # idx bounce buffer
    idxw = nc.dram_tensor("idxw", [NT, 1024], i16)        # wrapped idx bounce

    with tile.TileContext(nc) as tc, ExitStack() as ctx:
        wpool = ctx.enter_context(tc.tile_pool(name="weights", bufs=1))
        hpool = ctx.enter_context(tc.tile_pool(name="acts", bufs=2))
        spool = ctx.enter_context(tc.tile_pool(name="scores", bufs=2))
        gpool = ctx.enter_context(tc.tile_pool(name="gath", bufs=2))
        ipool = ctx.enter_context(tc.tile_pool(name="idx", bufs=3))
        smol = ctx.enter_context(tc.tile_pool(name="small", bufs=4))
        opool = ctx.enter_context(tc.tile_pool(name="outs", bufs=4))
        tpool = ctx.enter_context(tc.tile_pool(name="ttok", bufs=1))
        ps_s = ctx.enter_context(tc.tile_pool(name="ps_s", bufs=2, space="PSUM"))
        ps_m = ctx.enter_context(tc.tile_pool(name="ps_m", bufs=2, space="PSUM"))
        ps_w = ctx.enter_context(tc.tile_pool(name="ps_w", bufs=1, space="PSUM"))
        ps_t = ctx.enter_context(tc.tile_pool(name="ps_t", bufs=2, space="PSUM"))

        nc.gpsimd.load_library(library_config.mlp)
        nidx_reg = nc.gpsimd.to_reg(1024)

        def load(pool, param, dtype=None, shape=None):
            t = pool.tile(shape or list(param.shape), dtype or param.dtype,
                          name=param.name + "_sb", tag=param.name + "_sb")
            nc.sync.dma_start(t[:], param[:])
            return t

        # ---- resident tiles ----
        ident_sb = load(wpool, ident)
        tin_sb = load(wpool, tin)
        wt1 = load(wpool, Wt1)

        def load2(param, name):
            ts = []
            for kc in range(2):
                t = wpool.tile([128, D], bf, tag=f"{name}{kc}", name=f"{name}{kc}")
                nc.sync.dma_start(t[:], param[kc * 128:(kc + 1) * 128, :])
                ts.append(t)
            return ts

        wt2 = load2(Wt2, "wt2")
        wt3 = load2(Wt3, "wt3")
        bt1 = load(wpool, bt1w)
        bt2 = load(wpool, bt2w)
        bto = load(wpool, btow)
        btab = load(wpool, btabw)

        # =============== Phase T: tactile tokens ===============
        h1t = [tpool.tile([128, M], bf, tag=f"h1t{d}", name=f"h1t{d}") for d in range(2)]
        h2t = [tpool.tile([128, M], bf, tag=f"h2t{d}", name=f"h2t{d}") for d in range(2)]
        ttok_out = [tpool.tile([128, M], f32, tag=f"tto{d}", name=f"tto{d}") for d in range(2)]
        tabf = [tpool.tile([128, M], bf, tag=f"tab{d}", name=f"tab{d}") for d in range(2)]

        for q in range(M // 512):
            sl = slice(q * 512, (q + 1) * 512)
            for dc in range(2):
                ps = ps_m.tile([128, 512], f32)
                nc.tensor.matmul(ps[:], wt1[:, dc * 128:(dc + 1) * 128], tin_sb[:, sl],
                                 start=True, stop=True)
                nc.scalar.activation(h1t[dc][:, sl], ps[:], AF.Gelu,
                                     bias=bt1[:, dc:dc + 1], scale=1.0)
        for q in range(M // 512):
            sl = slice(q * 512, (q + 1) * 512)
            for dc in range(2):
                ps = ps_m.tile([128, 512], f32)
                for kc in range(2):
                    nc.tensor.matmul(ps[:], wt2[kc][:, dc * 128:(dc + 1) * 128],
                                     h1t[kc][:, sl], start=(kc == 0), stop=(kc == 1))
                nc.scalar.activation(h2t[dc][:, sl], ps[:], AF.Gelu,
                                     bias=bt2[:, dc:dc + 1], scale=1.0)
        for q in range(M // 512):
            sl = slice(q * 512, (q + 1) * 512)
            for dc in range(2):
                ps = ps_m.tile([128, 512], f32)
                for kc in range(2):
                    nc.tensor.matmul(ps[:], wt3[kc][:, dc * 128:(dc + 1) * 128],
                                     h2t[kc][:, sl], start=(kc == 0), stop=(kc == 1))
                # output rows: ttok + bt3 + ctx (f32); table: + bp3 as well (bf16)
                nc.vector.tensor_scalar(ttok_out[dc][:, sl], ps[:], bto[:, dc:dc + 1], None, ALU.add)
                nc.vector.tensor_scalar(tabf[dc][:, sl], ps[:], btab[:, dc:dc + 1], None, ALU.add)

        for dc in range(2):
            nc.sync.dma_start(out[dc * 128:(dc + 1) * 128, N:N + M], ttok_out[dc][:])

        # gather table: transpose [feat, tok] -> ttok_rm [tok, feat] (bf16)
        for mc in range(M // 128):
            for dc in range(2):
                pst = ps_t.tile([128, 128], bf)
                nc.tensor.transpose(pst[:], tabf[dc][:, mc * 128:(mc + 1) * 128], ident_sb[:])
                stg = opool.tile([128, 128], bf, tag="tabstg")
                nc.scalar.activation(stg[:], pst[:], AF.Copy)
                nc.sync.dma_start(ttok_rm[mc * 128:(mc + 1) * 128, dc * 128:(dc + 1) * 128], stg[:])

        # =============== Phase P: points ===============
        pinA_sb = load(wpool, pinA)
        pinB_sb = load(wpool, pinB)
        pt4_sb = load(wpool, pt4)
        tt4_sb = load(wpool, tt4)
        pn_sb = load(wpool, pnw)
        wp1a = load(wpool, Wp1a)
        wp1b = load(wpool, Wp1b)
        wp2 = load2(Wp2, "wp2")
        wp3 = load2(Wp3, "wp3")
        bp1 = load(wpool, bp1w)
        bp2 = load(wpool, bp2w)

        for rep_ch in range(repeat * (N // 512)):
            ch = rep_ch % (N // 512)
            csl = slice(ch * 512, (ch + 1) * 512)
            h1p = [hpool.tile([128, 512], bf, tag=f"h1p{d}", name=f"h1p{d}") for d in range(2)]
            for dc in range(2):
                ps = ps_m.tile([128, 512], f32)
                nc.tensor.matmul(ps[:], wp1a[:, dc * 128:(dc + 1) * 128], pinA_sb[:, csl],
                                 start=True, stop=False)
                nc.tensor.matmul(ps[:], wp1b[:, dc * 128:(dc + 1) * 128], pinB_sb[:, csl],
                                 start=False, stop=True)
                nc.scalar.activation(h1p[dc][:], ps[:], AF.Gelu, bias=bp1[:, dc:dc + 1], scale=1.0)
            h2p = [hpool.tile([128, 512], bf, tag=f"h2p{d}", name=f"h2p{d}") for d in range(2)]
            for dc in range(2):
                ps = ps_m.tile([128, 512], f32)
                for kc in range(2):
                    nc.tensor.matmul(ps[:], wp2[kc][:, dc * 128:(dc + 1) * 128],
                                     h1p[kc][:], start=(kc == 0), stop=(kc == 1))
                nc.scalar.activation(h2p[dc][:], ps[:], AF.Gelu, bias=bp2[:, dc:dc + 1], scale=1.0)

            for sub in range(4):
                t = ch * 4 + sub
                tsl = slice(t * 128, (t + 1) * 128)
                ssl = slice(sub * 128, (sub + 1) * 128)

                # ---- scores: [128 pts, M] ----
                s_sb = spool.tile([128, M], f32, tag="s_sb")
                for q in range(M // 512):
                    ps = ps_s.tile([128, 512], f32)
                    nc.tensor.matmul(ps[:], pt4_sb[:, tsl], tt4_sb[:, q * 512:(q + 1) * 512],
                                     start=True, stop=True)
                    nc.scalar.activation(s_sb[:, q * 512:(q + 1) * 512], ps[:], AF.Copy)

                # ---- top-8 (largest score == nearest) ----
                v8 = smol.tile([128, 8], f32, tag="v8")
                nc.vector.max(v8[:], s_sb[:])
                i8 = smol.tile([128, 8], u16, tag="i8")
                nc.vector.max_index(i8[:], v8[:], s_sb[:])

                # ---- softmax(-d/T) weights ----
                d2 = smol.tile([128, 8], f32, tag="d2")
                nc.vector.tensor_scalar(d2[:], v8[:], -2.0, pn_sb[:, t:t + 1], ALU.mult, ALU.add)
                nc.vector.tensor_scalar_max(d2[:], d2[:], 0.0)
                dd = smol.tile([128, 8], f32, tag="dd")
                nc.scalar.activation(dd[:], d2[:], AF.Sqrt)
                uu = smol.tile([128, 8], f32, tag="uu")
                nc.vector.tensor_scalar(uu[:], dd[:], dd[:, 0:1], None, ALU.subtract)
                ee = smol.tile([128, 8], f32, tag="ee")
                nc.scalar.activation(ee[:], uu[:], AF.Exp, scale=-1.0 / TAC_TEMP)
                zz = smol.tile([128, 1], f32, tag="zz")
                nc.vector.reduce_sum(zz[:], ee[:], axis=AX.X)
                rz = smol.tile([128, 1], f32, tag="rz")
                nc.vector.reciprocal(rz[:], zz[:])
                ww = smol.tile([128, 8], f32, tag="ww")
                nc.vector.tensor_scalar(ww[:], ee[:], rz[:, 0:1], None, ALU.mult)

                # ---- indices -> wrapped gather layout ----
                nc.sync.dma_start(idxd[t], i8[:].bitcast(i16))
                idx_sb = ipool.tile([128, 64], i16, tag="idx")
                tmpi = ipool.tile([128, 8], i16, tag="tmpi")
                nc.sync.dma_start_transpose(tmpi[:], idxd[t].flatten().rearrange("(b c) -> b c", b=8))
                nc.sync.dma_start(idxw[t].rearrange("(p j) -> p j", p=128), tmpi[:])
                rep = idxw[t]
                rep = bass.AP(tensor=rep.tensor, offset=rep.offset,
                              ap=[[0, 8]] + list(rep.ap))
                nc.sync.dma_start(idx_sb[:], rep)

                # ---- gather tokens [128, 8, 256] bf16 ----
                G = gpool.tile([128, 8, D], bf, tag="G")
                nc.gpsimd.dma_gather(G[:], ttok_rm[:, :], idx_sb[:], num_idxs=1024,
                                     num_idxs_reg=nidx_reg, elem_size=D,
                                     queue_num=t % 4)

                # ---- weighted sum via identity matmuls ----
                Gw = gpool.tile([128, 8, D], bf, tag="Gw")
                for c in range(8):
                    nc.scalar.activation(Gw[:, c, :], G[:, c, :], AF.Copy, scale=ww[:, c:c + 1])
                ps_r = ps_w.tile([128, D], f32)
                for c in range(8):
                    nc.tensor.matmul(ps_r[:], ident_sb[:], Gw[:, c, :],
                                     start=(c == 0), stop=(c == 7))
                treg = opool.tile([128, D], bf, tag="treg")
                nc.scalar.activation(treg[:], ps_r[:], AF.Copy)

                # ---- point L3 + treg^T -> output ----
                for dc in range(2):
                    ps3 = ps_m.tile([128, 128], f32, tag="ps")
                    for kc in range(2):
                        nc.tensor.matmul(ps3[:], wp3[kc][:, dc * 128:(dc + 1) * 128],
                                         h2p[kc][:, ssl], start=(kc == 0), stop=(kc == 1))
                    pst = ps_t.tile([128, 128], bf)
                    nc.tensor.transpose(pst[:], treg[:, dc * 128:(dc + 1) * 128], ident_sb[:])
                    tt_sb = opool.tile([128, 128], f32, tag="tt_sb")
                    nc.scalar.activation(tt_sb[:], pst[:], AF.Copy)
                    o_sb = opool.tile([128, 128], f32, tag="o_sb")
                    nc.vector.tensor_tensor(o_sb[:], ps3[:], tt_sb[:], ALU.add)
                    nc.sync.dma_start(out[dc * 128:(dc + 1) * 128, tsl], o_sb[:])

    _split_sync_waits(nc)
    from concourse.library_overlay import lower_extended_insts
    lower_extended_insts(nc)
    return nc


def _split_sync_waits(nc, maxw=1):
    """This walrus build rejects instructions carrying several sem-waits
    ("Too many sync wait commands").  Hoist excess waits onto standalone
    event-semaphore instructions just before the carrier."""
    from concourse import mybir
    k = 0
    for f in nc.m.functions:
        for bb in f.blocks:
            insts = list(bb.instructions)
            out = []
            changed = False
            for inst in insts:
                si = inst.sync_info
                waits = list(si.on_wait) if si is not None and si.on_wait else []
                if len(waits) > maxw:
                    for w in waits[:-maxw]:
                        k += 1
                        ev = mybir.InstEventSemaphore(name=f"wsplit_{k}", ins=[], outs=[])
                        ev.engine = inst.engine
                        ev.sync_info = mybir.SyncInfo(on_wait=[w], on_update=[])
                        out.append(ev)
                    si.on_wait = waits[-maxw:]
                    changed = True
                out.append(inst)
            if changed:
                bb.instructions = out


def _host_prep(inputs):
    """Build per-core input maps from the full inputs."""
    f32 = np.float32
    p_xyz = np.asarray(inputs["point_xyz_norm"], f32)
    p_feat = np.asarray(inputs["point_feats"], f32)
    t_xyz = np.asarray(inputs["tactile_xyz_norm"], f32)
    t_feat = np.asarray(inputs["tactile_feats"], f32)
    tri = np.asarray(inputs["triplane_feats_at_points"], f32)
    ctx = np.asarray(inputs["ctx_emb"], f32)
    W = {k: np.asarray(inputs[k], f32) for k in
         ("Wp1", "bp1", "Wp2", "bp2", "Wp3", "bp3", "Wt1", "bt1", "Wt2", "bt2", "Wt3", "bt3")}

    def wrap_bias(v):  # [256] -> [128, 2]
        return np.ascontiguousarray(v.reshape(2, 128).T)

    ident = np.eye(128, dtype=BF16)
    in_maps = []
    for b in range(B):
        pe_p = _pe3_np(p_xyz[b])                      # [N, 36]
        point_in = np.concatenate([p_feat[b], pe_p, tri[b]], axis=1)   # [N, 138]
        pin_T = np.ascontiguousarray(point_in.T)      # [138, N]
        pe_t = _pe3_np(t_xyz[b])
        tac_in = np.ascontiguousarray(
            np.concatenate([t_feat[b], pe_t], axis=1).T)               # [52, M]

        pt4 = np.concatenate([p_xyz[b].T, np.ones((1, N), f32)], 0)    # [4, N]
        tt4 = np.concatenate([t_xyz[b].T,
                              -0.5 * np.sum(t_xyz[b] ** 2, 1)[None, :]], 0)  # [4, M]
        pn = np.sum(p_xyz[b] ** 2, 1).reshape(NT, 128).T               # [128, NT]

        m = {
            "pinA": pin_T[:128].astype(BF16),
            "pinB": np.ascontiguousarray(pin_T[128:]).astype(BF16),
            "tin": tac_in.astype(BF16),
            "pt4": np.ascontiguousarray(pt4),
            "tt4": np.ascontiguousarray(tt4),
            "pnw": np.ascontiguousarray(pn),
            "Wp1a": W["Wp1"][:128].astype(BF16),
            "Wp1b": np.ascontiguousarray(W["Wp1"][128:]).astype(BF16),
            "Wp2": W["Wp2"].astype(BF16),
            "Wp3": W["Wp3"].astype(BF16),
            "Wt1": W["Wt1"].astype(BF16),
            "Wt2": W["Wt2"].astype(BF16),
            "Wt3": W["Wt3"].astype(BF16),
            "bp1w": wrap_bias(W["bp1"]),
            "bp2w": wrap_bias(W["bp2"]),
            "bt1w": wrap_bias(W["bt1"]),
            "bt2w": wrap_bias(W["bt2"]),
            "btow": wrap_bias(W["bt3"] + ctx[b]),
            "btabw": wrap_bias(W["bt3"] + ctx[b] + W["bp3"]),
            "ident": ident,
        }
        in_maps.append(m)
    return in_maps


def kernel(**inputs):
    from concourse.bass_utils import run_bass_kernel_spmd

    if "nc" not in _NC_CACHE:
        _NC_CACHE["nc"] = _build_nc()
    nc = _NC_CACHE["nc"]

    import os
    in_maps = _host_prep(inputs)
    trace = bool(int(os.environ.get("KERNEL_TRACE", "0")))
    res = run_bass_kernel_spmd(nc, in_maps, core_ids=list(range(B)), trace=trace)
    _NC_CACHE["last_result"] = res

    ctx = np.asarray(inputs["ctx_emb"], np.float32)
    gtok = np.asarray(inputs["global_token"], np.float32).reshape(D)
    out = np.empty((B, 1 + N + M, D), np.float32)
    for b in range(B):
        fm = np.asarray(res.results[b]["out"])       # [D, N+M]
        out[b, 0] = gtok + ctx[b]
        out[b, 1:] = fm.T
    return out


def benchmark(inputs, iters=20):
    """Time repeated on-device executions (inputs pre-staged, no donation)."""
    import time
    import jax
    import jax.numpy as jnp
    from jax.sharding import Mesh, PartitionSpec
    from jax.experimental.shard_map import shard_map
    from concourse import bass2jax as b2j

    if "nc" not in _NC_CACHE:
        _NC_CACHE["nc"] = _build_nc()
    nc = _NC_CACHE["nc"]
    b2j.install_neuronx_cc_hook()

    in_maps = _host_prep(inputs)
    from concourse import mybir
    in_names, out_names, out_avals, zero_outs = [], [], [], []
    partition_name = nc.partition_id_tensor.name if nc.partition_id_tensor else None
    for alloc in nc.m.functions[0].allocations:
        if not isinstance(alloc, mybir.MemoryLocationSet):
            continue
        name = alloc.memorylocations[0].name
        if alloc.kind == "ExternalInput":
            if name != partition_name:
                in_names.append(name)
        elif alloc.kind == "ExternalOutput":
            out_names.append(name)
            shape = list(alloc.tensor_shape)
            np_dt = np.dtype(mybir.dt.np(alloc.dtype))
            out_avals.append(jax.core.ShapedArray(shape, np_dt))
            zero_outs.append(np.zeros(shape, np_dt))
    n_params = len(in_names)
    all_in_names = list(in_names) + out_names
    if partition_name is not None:
        all_in_names.append(partition_name)

    def _body(*args):
        operands = list(args)
        if partition_name is not None:
            operands.append(b2j.partition_id_tensor())
        outs = b2j._bass_exec_p.bind(
            *operands, out_avals=tuple(out_avals), in_names=tuple(all_in_names),
            out_names=tuple(out_names), lowering_input_output_aliases=(),
            sim_require_finite=True, sim_require_nnan=True, nc=nc)
        return tuple(outs)

    devices = jax.devices()[:B]
    mesh = Mesh(np.asarray(devices), ("core",))
    nio = n_params + len(out_names)
    fn = jax.jit(shard_map(_body, mesh=mesh,
                           in_specs=(PartitionSpec("core"),) * nio,
                           out_specs=(PartitionSpec("core"),) * len(out_names),
                           check_rep=False), keep_unused=True)
    concat_in = [np.concatenate([np.asarray(in_maps[c][n]) for c in range(B)], axis=0)
                 for n in in_names]
    concat_zeros = [np.zeros((B * z.shape[0], *z.shape[1:]), z.dtype) for z in zero_outs]
    from jax.sharding import NamedSharding
    sh = NamedSharding(mesh, PartitionSpec("core"))
    dev_in = [jax.device_put(x, sh) for x in concat_in + concat_zeros]
    _NC_CACHE["bench_fn"] = (fn, dev_in)
    outs = fn(*dev_in)
    jax.block_until_ready(outs)
    times = []
    for _ in range(iters):
        t0 = time.perf_counter()
        outs = fn(*dev_in)
        jax.block_until_ready(outs)
        times.append(time.perf_counter() - t0)
    return min(times), times


def benchmark_pipelined(inputs, n_lo=100, n_hi=200):
    """Marginal per-call time from pipelined async dispatches: amortizes the
    axon round-trip latency; returns (T(n_hi)-T(n_lo))/(n_hi-n_lo) seconds."""
    import time
    import jax
    # benchmark() must have been called first (compiles + stages buffers)
    best, _ = benchmark(inputs, iters=1)
    fn, dev_in = _NC_CACHE["bench_fn"]
    out = None
    ts = {}
    for n in (n_lo, n_hi):
        t0 = time.perf_counter()
        outs = [fn(*dev_in) for _ in range(n)]
        jax.block_until_ready(outs)
        ts[n] = time.perf_counter() - t0
    return (ts[n_hi] - ts[n_lo]) / (n_hi - n_lo)

